# revision 11
# baseline (speedup 1.0000x reference)
"""Trainium2 Bass kernel for nn_BatchMuSc (retrieval_knn).

Computes, for Z [96, 256, 128] and cls_tokens [96, 768]:
  - MSM patch anomaly scores: for each image i, for each of its 256 patches,
    the mean of the 28 smallest per-reference-image minimal euclidean
    distances to all other images' patches.
  - img_scores = max over patches; min-max normalize.
  - RsCIN/MMO refinement with W = cls @ cls.T, top-k row masks (k=1,2,3).
  Output: [96] float32.

Strategy (8 NeuronCores, data-parallel over query images):
  - Every core receives the full Z, rolled by -12*core images, so its 12
    query images are always local images 0..11 (static addressing; SPMD).
  - Per core: ZT [128(C), 24576] resident in SBUF (fp32r), distances via
    PSUM-accumulated fp32r matmuls: B = 2*q.z - |z|^2 (rank-1 adds -|z|^2),
    grouped max-reduce over each reference image's 256 patches
    (max B = -min d2), then top-28-smallest via max8/match_replace.
  - img_scores are AllGathered across cores; every core redundantly runs the
    tiny MMO refinement; core 0's output is returned.
"""
import os
import sys
import types

import numpy as np

for _p in ("/opt/trn_rl_repo",):
    if _p not in sys.path and os.path.isdir(_p):
        sys.path.insert(0, _p)

# The axon NTFF profile hook module is absent in this environment; stub it so
# run_bass_kernel_spmd can import it (only needed for trace=True).
try:  # pragma: no cover
    import antenv.axon_hooks  # noqa: F401
except Exception:  # pragma: no cover
    _m = types.ModuleType("antenv.axon_hooks")
    _m.get_axon_ntff_profile_hook = lambda: None
    sys.modules["antenv.axon_hooks"] = _m

import concourse.bacc as bacc
import concourse.bass_isa as bass_isa
import concourse.mybir as mybir
from concourse import bass_utils
from concourse.masks import make_identity
from concourse.tile import TileContext

F32 = mybir.dt.float32
F32R = mybir.dt.float32r
AX = mybir.AxisListType.X
OP = mybir.AluOpType
ACTF = mybir.ActivationFunctionType

N, L, C, DC = 96, 256, 128, 768
NCORES = 8
IPC = N // NCORES          # 12 query images per core
NL = N * L                 # 24576 total patches
NT = NL // 128             # 192 transpose tiles
NS = NL // 512             # 48 stripes of 512 patches (2 images each)
NQ = NS // 4               # 12 quads of 4 stripes (8 images each)
KTOP = 28                  # int((N-1)*0.3) smallest distances averaged
EPS = 1e-12
NEG = -3.4e38

# Per-quad reduction path: D = DVE reduce from PSUM;
# A = ACT copy to SBUF + DVE reduce; G = ACT copy + GPSIMD pairwise fold +
# DVE short reduce.  Tunable (len NQ).
QUAD_PATHS = os.environ.get("BMS_QUAD_PATHS", "AAAAAAAAAAAA")
assert len(QUAD_PATHS) == NQ and set(QUAD_PATHS) <= set("DAG")


def build(
    quad_paths: str = QUAD_PATHS,
    repeat_main: int = 1,
    n_cores: int = NCORES,
    stop: str = "full",
):
    nc = bacc.Bacc(
        "TRN2",
        target_bir_lowering=False,
        debug=False,
        enable_asserts=False,
        num_devices=n_cores,
    )
    Z = nc.dram_tensor("Z", [N, L, C], F32, kind="ExternalInput")
    cls = nc.dram_tensor("cls_tokens", [N, DC], F32, kind="ExternalInput")
    out = nc.dram_tensor("out", [N], F32, kind="ExternalOutput")
    cc_in = nc.dram_tensor("cc_in", [IPC], F32, kind="Internal")
    cc_out = nc.dram_tensor("cc_out", [N], F32, kind="Internal", addr_space="Shared")

    stages = ["p0", "p0b", "p1", "p2", "full"]
    sidx = stages.index(stop)
    with TileContext(nc) as tc:
        with tc.tile_pool(name="persist", bufs=1) as pers:
            ident = pers.tile([128, 128], F32)
            make_identity(nc, ident)
            ones_f = pers.tile([128, 128], F32)
            nc.vector.memset(ones_f, 1.0)
            ones_r = pers.tile([128, 128], F32R)
            nc.vector.tensor_copy(ones_r, ones_f)
            epsb = pers.tile([128, 1], F32)
            nc.vector.memset(epsb, EPS)

            ZT = pers.tile([128, NL], F32R)          # channels x patches
            sq_q = pers.tile([128, 2 * IPC], F32)    # |z|^2 of local queries
            # -|z_p|^2 packed for rank-1 matmul rhs reads. Matmul operands
            # must start at partition 0/32/64, so stripes live on exactly
            # those three rows: row 32*(s//16), columns 512*(s%16).
            nsq = pers.tile([65, 16 * 512], F32R)
            score_all = pers.tile([128, 2 * IPC], F32)
            simg = pers.tile([1, N], F32)

            # ---- Phase 0: load Z, build ZT (transposed, fp32r), query norms
            Zf = Z.ap().rearrange("n l c -> (n l) c")
            with (
                tc.tile_pool(name="stage", bufs=4) as stage,
                tc.tile_pool(name="tpsum", bufs=8, space="PSUM") as tps,
                tc.tile_pool(name="sqscr", bufs=2) as sqscr,
            ):
                for t in range(NT):
                    st = stage.tile([128, C], F32, tag="st")
                    nc.sync.dma_start(st, Zf[128 * t : 128 * (t + 1), :])
                    pt = tps.tile([128, 128], F32, tag="pt")
                    nc.tensor.transpose(pt, st, ident)
                    dst = ZT[:, 128 * t : 128 * (t + 1)]
                    if t % 2 == 0:
                        nc.scalar.copy(dst, pt)
                    else:
                        nc.vector.tensor_copy(dst, pt)
                    if t < 2 * IPC:
                        dm = sqscr.tile([128, C], F32, tag="dm")
                        nc.scalar.activation(
                            dm, st, ACTF.Square, accum_out=sq_q[:, t : t + 1]
                        )

            # ---- Phase 0b: negated patch norms -|z_p|^2 in rank-1 layout
            if sidx >= 1:
              with (
                tc.tile_pool(name="z2p", bufs=3) as z2p,
                tc.tile_pool(name="sqpsum", bufs=4, space="PSUM") as sqp,
              ):
                for s in range(NS):
                    z2 = z2p.tile([128, 512], F32R, tag="z2")
                    nc.vector.tensor_mul(
                        z2, ZT[:, 512 * s : 512 * (s + 1)], ZT[:, 512 * s : 512 * (s + 1)]
                    )
                    psq = sqp.tile([128, 512], F32, tag="psq")
                    nc.tensor.matmul(psq, lhsT=ones_r, rhs=z2, start=True, stop=True)
                    # every psq row holds the same column sums; copy from the
                    # partition row matching nsq's layout
                    row = 32 * (s // 16)
                    off = 512 * (s % 16)
                    nc.scalar.mul(
                        nsq[row : row + 1, off : off + 512],
                        psq[row : row + 1, :],
                        -1.0,
                    )

            # ---- Phase 1: distances + per-image minima + top-28 means
            if sidx >= 2:
              with (
                tc.tile_pool(name="q2p", bufs=2) as q2p,
                tc.tile_pool(name="quadp", bufs=2, space="PSUM") as quadp,
                tc.tile_pool(name="cpp", bufs=3) as cpp,
                tc.tile_pool(name="foldp", bufs=2) as foldp,
                tc.tile_pool(name="maxbp", bufs=2) as maxbp,
                tc.tile_pool(name="smallp", bufs=2) as smallp,
              ):
                for _rep in range(repeat_main):
                    for i in range(IPC):
                        q2 = q2p.tile([128, L], F32R, tag="q2")
                        nc.scalar.mul(q2, ZT[:, L * i : L * (i + 1)], 2.0)
                        for h in range(2):
                            lhsT = q2[:, 128 * h : 128 * (h + 1)]
                            maxB = maxbp.tile([128, N], F32, tag="maxB")
                            for qd in range(NQ):
                                quad = quadp.tile([128, 4, 512], F32, tag="quad")
                                for j in range(4):
                                    s = 4 * qd + j
                                    row = 32 * (s // 16)
                                    off = 512 * (s % 16)
                                    nc.tensor.matmul(
                                        quad[:, j, :],
                                        lhsT=ones_r[row : row + 1, :],
                                        rhs=nsq[row : row + 1, off : off + 512],
                                        start=True,
                                        stop=False,
                                    )
                                for j in range(4):
                                    s = 4 * qd + j
                                    nc.tensor.matmul(
                                        quad[:, j, :],
                                        lhsT=lhsT,
                                        rhs=ZT[:, 512 * s : 512 * (s + 1)],
                                        start=False,
                                        stop=True,
                                    )
                                path = quad_paths[qd]
                                mslice = maxB[:, 8 * qd : 8 * qd + 8]
                                qflat = quad.rearrange("p f x -> p (f x)")
                                if path == "D":
                                    nc.vector.tensor_reduce(
                                        mslice,
                                        qflat.rearrange("p (g x) -> p g x", g=8),
                                        axis=AX,
                                        op=OP.max,
                                    )
                                else:
                                    cp = cpp.tile([128, 2048], F32, tag="cp")
                                    nc.scalar.copy(cp, qflat)
                                    if path == "A":
                                        nc.vector.tensor_reduce(
                                            mslice,
                                            cp.rearrange("p (g x) -> p g x", g=8),
                                            axis=AX,
                                            op=OP.max,
                                        )
                                    else:
                                        cpv = cp.rearrange(
                                            "p (g two x) -> p g two x", g=8, two=2
                                        )
                                        fold = foldp.tile([128, 1024], F32, tag="fold")
                                        fv = fold.rearrange(
                                            "p (g x) -> p g x", g=8
                                        )
                                        nc.gpsimd.tensor_tensor(
                                            fv,
                                            cpv[:, :, 0, :],
                                            cpv[:, :, 1, :],
                                            op=OP.max,
                                        )
                                        nc.vector.tensor_reduce(
                                            mslice, fv, axis=AX, op=OP.max
                                        )
                            # finalize (i, h): x = min(maxB - sq_q, 0) = -d2c
                            col = 2 * i + h
                            x = smallp.tile([128, N], F32, tag="x")
                            nc.vector.tensor_scalar(
                                x,
                                maxB,
                                sq_q[:, col : col + 1],
                                0.0,
                                op0=OP.subtract,
                                op1=OP.min,
                            )
                            nc.vector.memset(x[:, i : i + 1], NEG)
                            b8 = smallp.tile([128, 32], F32, tag="b8")
                            for r in range(4):
                                nc.vector.max(b8[:, 8 * r : 8 * r + 8], x)
                                if r < 3:
                                    nc.vector.match_replace(
                                        x,
                                        in_to_replace=b8[:, 8 * r : 8 * r + 8],
                                        in_values=x,
                                        imm_value=NEG,
                                    )
                            sv = smallp.tile([128, KTOP], F32, tag="sv")
                            nc.scalar.activation(
                                sv, b8[:, 0:KTOP], ACTF.Sqrt, bias=epsb, scale=-1.0
                            )
                            s28 = smallp.tile([128, 1], F32, tag="s28")
                            nc.vector.reduce_sum(s28, sv, axis=AX)
                            nc.vector.tensor_scalar(
                                score_all[:, col : col + 1],
                                s28,
                                1.0 / KTOP,
                                None,
                                op0=OP.mult,
                            )

            # ---- Phase 2: image scores + AllGather
            if sidx >= 3:
              with tc.tile_pool(name="p2", bufs=1) as p2:
                red = p2.tile([128, 2 * IPC], F32)
                nc.gpsimd.partition_all_reduce(
                    red, score_all, channels=128, reduce_op=bass_isa.ReduceOp.max
                )
                img12 = p2.tile([1, IPC], F32)
                nc.vector.tensor_reduce(
                    img12,
                    red[0:1, :].rearrange("p (i h) -> p i h", h=2),
                    axis=AX,
                    op=OP.max,
                )
                nc.sync.dma_start(cc_in.ap(), img12)
                nc.gpsimd.collective_compute(
                    "AllGather",
                    OP.bypass,
                    replica_groups=[list(range(NCORES))],
                    ins=[cc_in.ap()],
                    outs=[cc_out.ap()],
                )
                nc.sync.dma_start(simg, cc_out.ap())

            # ---- Phase 3: RsCIN / MMO (redundant on every core)
            if sidx >= 4:
              with (
                tc.tile_pool(name="p3", bufs=1) as p3,
                tc.tile_pool(name="p3psum", bufs=2, space="PSUM") as p3p,
              ):
                mn = p3.tile([1, 1], F32)
                mx = p3.tile([1, 1], F32)
                nc.vector.tensor_reduce(mn, simg, axis=AX, op=OP.min)
                nc.vector.tensor_reduce(mx, simg, axis=AX, op=OP.max)
                rngv = p3.tile([1, 1], F32)
                nc.vector.tensor_sub(rngv, mx, mn)
                rcp = p3.tile([1, 1], F32)
                nc.vector.reciprocal(rcp, rngv)
                s_norm = p3.tile([1, N], F32)
                nc.vector.tensor_scalar(
                    s_norm, simg, mn, rcp, op0=OP.subtract, op1=OP.mult
                )
                s_rep = p3.tile([N, N], F32)
                nc.gpsimd.partition_broadcast(s_rep, s_norm, channels=N)

                cls_sb = p3.tile([N, DC], F32)
                nc.sync.dma_start(cls_sb, cls.ap())
                clsT = p3.tile([128, DC // 128, N], F32)
                for d in range(DC // 128):
                    pt = p3p.tile([128, N], F32, tag="pt3")
                    nc.tensor.transpose(
                        pt, cls_sb[:, 128 * d : 128 * (d + 1)], ident[0:N, 0:N]
                    )
                    nc.scalar.copy(clsT[:, d, :], pt)
                Wp = p3p.tile([N, N], F32, tag="Wp")
                for d in range(DC // 128):
                    nc.tensor.matmul(
                        Wp,
                        lhsT=clsT[:, d, :],
                        rhs=clsT[:, d, :],
                        start=(d == 0),
                        stop=(d == DC // 128 - 1),
                    )
                W = p3.tile([N, N], F32)
                nc.scalar.copy(W, Wp)
                m8w = p3.tile([N, 8], F32)
                nc.vector.max(m8w, W)
                acc = p3.tile([N, 1], F32)
                nc.vector.memset(acc, 0.0)
                Wm = p3.tile([N, N], F32)
                Pk = p3.tile([N, N], F32)
                for k in (1, 2, 3):
                    rs = p3.tile([N, 1], F32, tag=f"rs{k}")
                    nc.vector.scalar_tensor_tensor(
                        out=Wm,
                        in0=W,
                        scalar=m8w[:, k - 1 : k],
                        in1=W,
                        op0=OP.is_ge,
                        op1=OP.mult,
                        accum_out=rs,
                    )
                    rck = p3.tile([N, 1], F32, tag=f"rck{k}")
                    nc.vector.reciprocal(rck, rs)
                    Sk = p3.tile([N, 1], F32, tag=f"Sk{k}")
                    nc.vector.tensor_mul(Pk, Wm, s_rep)
                    nc.vector.reduce_sum(Sk, Pk, axis=AX)
                    term = p3.tile([N, 1], F32, tag=f"term{k}")
                    nc.vector.tensor_scalar(term, Sk, rck, None, op0=OP.mult)
                    nc.vector.tensor_add(acc, acc, term)
                out_sb = p3.tile([N, 1], F32)
                nc.vector.tensor_scalar(
                    out_sb, acc, 1.0 / 3.0, None, op0=OP.mult
                )
                nc.sync.dma_start(out.ap(), out_sb)
            if sidx < 4:
                with tc.tile_pool(name="dbg", bufs=1) as dbg:
                    dt_ = dbg.tile([1, N], F32)
                    src_ap = score_all[0:1, 0:24] if sidx >= 2 else ZT[0:1, 0:24]
                    nc.vector.tensor_scalar(
                        dt_[:, 0:24], src_ap.bitcast(F32), 1.0, None, op0=OP.mult
                    )
                    nc.vector.memset(dt_[:, 24:N], 0.0)
                    nc.sync.dma_start(out.ap(), dt_)

    nc.finalize()
    return nc


_CACHE: dict = {}


def _get_nc():
    key = (QUAD_PATHS,)
    if key not in _CACHE:
        _CACHE[key] = build(QUAD_PATHS)
    return _CACHE[key]


def kernel(Z: np.ndarray, cls_tokens: np.ndarray) -> np.ndarray:
    assert Z.shape == (N, L, C) and cls_tokens.shape == (N, DC)
    Z = np.ascontiguousarray(Z, dtype=np.float32)
    cls_tokens = np.ascontiguousarray(cls_tokens, dtype=np.float32)
    nc = _get_nc()
    in_maps = [
        {"Z": np.ascontiguousarray(np.roll(Z, -IPC * c, axis=0)), "cls_tokens": cls_tokens}
        for c in range(NCORES)
    ]
    res = bass_utils.run_bass_kernel_spmd(nc, in_maps, core_ids=list(range(NCORES)))
    return np.asarray(res.results[0]["out"], dtype=np.float32)


if __name__ == "__main__":
    rng = np.random.default_rng(0)
    Zv = rng.standard_normal((N, L, C), dtype=np.float32)
    cv = rng.standard_normal((N, DC), dtype=np.float32)
    print(kernel(Zv, cv)[:8])
